# revision 6
# baseline (speedup 1.0000x reference)
"""nn_AdapFilter3d Trainium2 kernel — 8-core SPMD (data-parallel over (B,C)).

out[b,c,z,y,x] = sum_{i,j,k} pad(input)[b,c,z+i-1,y+j-1,x+k-1] * F[b,c,z,y,x,i,j,k]

v2 strategy (per NeuronCore: 4 of the 32 (b,c) slices = 2 slice-pairs;
partitions p = 64*s + y; free dims carry (z, x) densely):

  - F streams from HBM as fp8 E3M4 (4 mantissa bits): halves the dominant
    HBM stream (28.3MB -> 14.2MB/core). Simulated end-to-end rel err
    1.36e-2 < 2e-2 gate (F range |F|<5.5 << 15.5 = e3m4 max, no clipping).
  - y-shift via accumulating matmuls with shift stationaries S_j
    (host pre-shifts F by -dy per j; S_j un-shifts the product), x/z
    shifts are free-dim offsets into one padded dense (z,x) row.
  - Taps regrouped into 9 slots s (j,k per slot, i inside): 4 "fused"
    slots multiply x_bf16 * F_fp8 directly on DVE (1x mode), 5 "upconvert"
    slots go ScalarE fp8->bf16 copy then DVE bf16 multiply (2x mode).
    This balances DVE (~60us) / ScalarE (~60us) / PE (~56us) / DMA (~55us)
    instead of bottlenecking any one engine.
  - DVE 2x alignment: window base for k-offset taps is 512*ch + k; odd
    (k=1) bases break 2x packing, so a second x copy shifted by +1
    element (same HBM buffer read at offset 1) gives even bases for k=1.
  - Per chunk: 27 accumulating TensorE matmuls into one PSUM bank;
    ScalarE evicts to bf16; output rides the scalar queue; F rides
    gpsimd/sync rings (whole-128-partition transfers).

Self-contained: hardcodes shapes from the problem spec.
"""

import time

import numpy as np

import bass_rust
import concourse.bacc as bacc
import concourse.tile as tile
from concourse import mybir
from concourse.bass_utils import run_bass_kernel_spmd

B, C, D, H, W = 2, 16, 32, 64, 64
BC = B * C
TAPS = 27
N_CORES = 8
S = BC // N_CORES  # 4 slices per core
PAIRS = S // 2  # 2
ZC = 8  # z planes per chunk
NCHUNK = D // ZC  # 4
FD = ZC * W  # 512
SFD = 3 * FD  # 1536 (one slot = 3 i-taps)
CFD = TAPS * FD  # 13824 (one chunk of F)
DW = D * W  # 2048 dense (z,x) elements per (slice, y)
FRONT = 65  # zero pad around the dense (z,x) block (>= W+1)
XPLEN = FRONT + DW + FRONT

# slot s holds taps (i=0..2, j=SLOT_J[s], k=SLOT_K[s]); slots 0-3 are
# DVE-fused fp8 multiplies, slots 4-8 are ScalarE-upconverted to bf16
SLOT_J = [2, 0, 1, 2, 0, 1, 2, 0, 1]
SLOT_K = [1, 2, 2, 2, 0, 0, 0, 1, 1]
NFUSED = 4

F32 = mybir.dt.float32
IO_DT = mybir.dt.bfloat16
F8 = mybir.dt.float8e3


def _overlap_ap(tile_ap, start, dims):
    """AP on tile_ap's tensor at element offset `start` with custom free dims
    [[stride, num], ...] (keeps the tile's partition dim)."""
    return bass_rust.AP(tile_ap.tensor, start, [list(tile_ap.ap[0])] + dims)


def _build():
    nc = bacc.Bacc()
    x_ext = nc.declare_dram_parameter("input", [PAIRS, 128, XPLEN], IO_DT, isOutput=False)
    f_ext = nc.declare_dram_parameter("F", [PAIRS, 128, NCHUNK * CFD], F8, isOutput=False)
    s_ext = nc.declare_dram_parameter("stat", [128, 3 * 128], IO_DT, isOutput=False)
    out_ext = nc.declare_dram_parameter("out", [PAIRS, 128, NCHUNK * FD], IO_DT, isOutput=True)

    with tile.TileContext(nc) as tc:
        with (
            tc.tile_pool(name="const", bufs=1) as cpool,
            tc.tile_pool(name="xp", bufs=2) as xpool,
            tc.tile_pool(name="fp", bufs=3) as fpool,
            tc.tile_pool(name="fb", bufs=2) as fbpool,
            tc.tile_pool(name="prod", bufs=2) as ppool,
            tc.tile_pool(name="osb", bufs=2) as opool,
            tc.tile_pool(name="ps", bufs=4, space="PSUM") as pspool,
        ):
            st = cpool.tile([128, 3 * 128], IO_DT)

            # x on the scalar queue, x2 on sync — parallel with the first F
            # chunk on gpsimd, so compute can start as early as possible
            xps, x2s = [], []
            for pair in range(PAIRS):
                xp = xpool.tile([128, XPLEN], IO_DT, tag="xp")
                nc.scalar.dma_start(xp[:, :], x_ext[pair, :, :])
                x2 = xpool.tile([128, XPLEN - 1], IO_DT, tag="x2")
                nc.sync.dma_start(x2[:, :], x_ext[pair, :, 1:XPLEN])
                xps.append(xp)
                x2s.append(x2)
            nc.scalar.dma_start(st[:], s_ext[:])

            pend = None  # lazily-evicted (psum, pair, ch)
            for it in range(PAIRS * NCHUNK):
                pair, ch = divmod(it, NCHUNK)
                xp, x2 = xps[pair], x2s[pair]
                ft = fpool.tile([128, CFD], F8, tag="ft")
                eng = nc.gpsimd if it % 2 == 0 else nc.sync
                base = ch * CFD
                first = it == 0
                # part1 = fused slots (DVE can start without ScalarE),
                # part2 = upconvert slots (feeds the ScalarE copy). The very
                # first chunk streams fused slots one-by-one so the pipeline
                # fills at slot granularity (~0.2MB instead of 0.8MB).
                if first:
                    for s in range(NFUSED):
                        eng.dma_start(
                            ft[:, s * SFD : (s + 1) * SFD],
                            f_ext[pair, :, base + s * SFD : base + (s + 1) * SFD],
                        )
                else:
                    eng.dma_start(
                        ft[:, : NFUSED * SFD], f_ext[pair, :, base : base + NFUSED * SFD]
                    )
                eng.dma_start(
                    ft[:, NFUSED * SFD :], f_ext[pair, :, base + NFUSED * SFD : base + CFD]
                )

                fb = fbpool.tile([128, (9 - NFUSED) * SFD], IO_DT, tag="fb")
                nc.scalar.copy(fb[:, :], ft[:, NFUSED * SFD :])

                prod = ppool.tile([128, CFD], IO_DT, tag="prod")
                psum = pspool.tile([128, FD], F32, tag="ps")
                xb = ch * FD  # even window base; +k for k in {0,2} on xp, x2 for k=1

                def slot_aps(t, s0, n):
                    return t[:, s0 * SFD : (s0 + n) * SFD].rearrange(
                        "p (s i e) -> p s i e", s=n, i=3
                    )

                def mm(s, i, start=False, stop=False):
                    t = 3 * s + i
                    nc.tensor.matmul(
                        psum[:],
                        st[:, SLOT_J[s] * 128 : (SLOT_J[s] + 1) * 128],
                        prod[:, t * FD : (t + 1) * FD],
                        start=start,
                        stop=stop,
                    )

                # DVE fused (fp8 operand, 1x mode): slot0 k=1 via x2, slots
                # 1-3 k=2 (merged with a stride-0 j-dim — fine at 1x). First
                # chunk issues per-slot to match the fill DMA granularity.
                def slot_ap2(t, s):
                    return t[:, s * SFD : (s + 1) * SFD].rearrange(
                        "p (i e) -> p i e", i=3
                    )

                if first:
                    for s in range(NFUSED):
                        src, sb = (x2, xb) if SLOT_K[s] == 1 else (xp, xb + SLOT_K[s])
                        nc.vector.tensor_mul(
                            slot_ap2(prod, s),
                            _overlap_ap(src[:], sb, [[W, 3], [1, FD]]),
                            slot_ap2(ft, s),
                        )
                        mm(s, 0, start=(s == 0))
                        mm(s, 1)
                        mm(s, 2)
                else:
                    nc.vector.tensor_mul(
                        prod[:, :SFD].rearrange("p (i e) -> p i e", i=3),
                        _overlap_ap(x2[:], xb, [[W, 3], [1, FD]]),
                        ft[:, :SFD].rearrange("p (i e) -> p i e", i=3),
                    )
                    nc.vector.tensor_mul(
                        slot_aps(prod, 1, 3),
                        _overlap_ap(xp[:], xb + 2, [[0, 3], [W, 3], [1, FD]]),
                        slot_aps(ft, 1, 3),
                    )
                    for s in range(NFUSED):
                        mm(s, 0, start=(s == 0))
                        mm(s, 1)
                        mm(s, 2)
                # DVE upconverted multiplies run 2x mode (bf16, step 1, even
                # 4B-aligned bases) — one instr per slot: a merged stride-0
                # broadcast dim on x was observed to force 1x mode.
                for s in range(NFUSED, 9):
                    src, sb = (x2, xb) if SLOT_K[s] == 1 else (xp, xb + SLOT_K[s])
                    nc.vector.tensor_mul(
                        slot_ap2(prod, s),
                        _overlap_ap(src[:], sb, [[W, 3], [1, FD]]),
                        fb[:, (s - NFUSED) * SFD : (s - NFUSED + 1) * SFD].rearrange(
                            "p (i e) -> p i e", i=3
                        ),
                    )
                    mm(s, 0)
                    mm(s, 1)
                    mm(s, 2, stop=(s == 8))
                # evict the PREVIOUS chunk after this chunk's fb copy is
                # queued, so the scalar queue never stalls the upconvert
                # behind a matmul-chain wait
                if pend is not None:
                    ppsum, ppair, pch = pend
                    osb = opool.tile([128, FD], IO_DT, tag="osb")
                    nc.scalar.copy(osb[:], ppsum[:])
                    nc.scalar.dma_start(
                        out_ext[ppair, :, pch * FD : (pch + 1) * FD], osb[:]
                    )
                pend = (psum, pair, ch)
            ppsum, ppair, pch = pend
            osb = opool.tile([128, FD], IO_DT, tag="osb")
            nc.scalar.copy(osb[:], ppsum[:])
            nc.scalar.dma_start(out_ext[ppair, :, pch * FD : (pch + 1) * FD], osb[:])
    nc.compile()
    return nc


_NC_CACHE = {}


def _host_inputs(input, F):
    """FULL inputs -> per-core in_maps with the kernel's layouts."""
    import ml_dtypes

    io_np = mybir.dt.np(IO_DT)
    f8_np = mybir.dt.np(F8)
    # x dense rows: xs[bc, y, FRONT + z*W + x]
    xs = np.zeros((BC, H, XPLEN), dtype=io_np)
    xs[:, :, FRONT : FRONT + DW] = (
        input.reshape(BC, D, H, W).transpose(0, 2, 1, 3).reshape(BC, H, DW).astype(io_np)
    )
    xs = xs.reshape(BC // 2, 128, XPLEN)

    # F pre-shifted along y by -dy per j, slot-ordered taps, edge taps zeroed
    base = np.ascontiguousarray(
        F.reshape(BC, D, H, W, 3, 3, 3).transpose(0, 2, 5, 4, 6, 1, 3)
    )  # [bc, y, j, i, k, z, x]
    Hs = np.zeros_like(base)
    Hs[:, : H - 1, 0] = base[:, 1:, 0]
    Hs[:, :, 1] = base[:, :, 1]
    Hs[:, 1:, 2] = base[:, : H - 1, 2]
    Hs[:, :, :, :, 0, :, 0] = 0
    Hs[:, :, :, :, 2, :, W - 1] = 0
    Hs[:, :, :, 0, :, 0, :] = 0
    Hs[:, :, :, 2, :, D - 1, :] = 0
    # slot-major: [bc, y, s, i, z, x]
    Hs = np.stack([Hs[:, :, SLOT_J[s], :, SLOT_K[s]] for s in range(9)], axis=2)
    fs = (
        Hs.reshape(BC, H, 9, 3, NCHUNK, ZC, W)
        .transpose(0, 1, 4, 2, 3, 5, 6)  # [bc, y, ch, s, i, zc, x]
        .reshape(BC // 2, 128, NCHUNK * CFD)
        .astype(f8_np)
    )

    # stationaries: st[kk, j*128+m] = 1 iff kk == m + (j-1), same 64-block
    stm = np.zeros((128, 3, 128), dtype=np.float32)
    for j in range(3):
        Sj = np.eye(128, k=-(j - 1), dtype=np.float32)
        Sj[0:64, 64:128] = 0
        Sj[64:128, 0:64] = 0
        stm[:, j, :] = Sj
    stm = stm.reshape(128, 3 * 128).astype(io_np)

    return [
        {
            "input": xs[c * PAIRS : (c + 1) * PAIRS],
            "F": fs[c * PAIRS : (c + 1) * PAIRS],
            "stat": stm,
        }
        for c in range(N_CORES)
    ]


def kernel(input: np.ndarray, F: np.ndarray) -> np.ndarray:
    input = np.asarray(input)
    F = np.asarray(F)
    assert input.shape == (B, C, D, H, W), input.shape
    assert F.shape == (B, C, D, H, W, 3, 3, 3), F.shape

    if "nc" not in _NC_CACHE:
        _NC_CACHE["nc"] = _build()
    nc = _NC_CACHE["nc"]

    in_maps = _host_inputs(input, F)
    # the fleet occasionally throws transient NRT_EXEC_UNIT_UNRECOVERABLE
    # device errors (observed in dev, cleared on retry)
    last_err = None
    for _attempt in range(3):
        try:
            res = run_bass_kernel_spmd(nc, in_maps, core_ids=list(range(N_CORES)))
            break
        except Exception as e:  # noqa: BLE001
            last_err = e
            time.sleep(2.0)
    else:
        raise last_err
    out = np.concatenate(
        [np.asarray(res.results[c]["out"], dtype=np.float32) for c in range(N_CORES)],
        axis=0,
    )  # [BC/2, 128, NCHUNK*FD]
    out = (
        out.reshape(BC // 2, 2, H, NCHUNK, ZC, W)
        .transpose(0, 1, 3, 4, 2, 5)  # [pair, s, ch, zc, y, x]
        .reshape(B, C, D, H, W)
        .astype(np.float32)
    )
    return np.ascontiguousarray(out)
